# revision 10
# baseline (speedup 1.0000x reference)
"""GCN layer (message passing + linear + BatchNorm) on 8 Trainium2 NeuronCores.

Strategy
--------
* Nodes are sharded contiguously across the 8 cores (12500 nodes each, 98
  chunks of 128 nodes).  Edges are partitioned by dst core so segment_sum is
  local; h is replicated (gathered directly from each core's HBM copy).
* h is pre-split on host into bf16 (hi, lo) pairs packed as h2[N,128] bf16 so
  one gathered 256B row reconstructs the fp32 value exactly
  (hi + lo == fp32 h to ~2^-17 relative).
* dma_gather uses int16 indices (max 32767), so src ids are classed by range
  (4 classes of 25000 rows); edges are bucketed (class, chunk), padded to
  128-edge tiles.
* Per 128-edge tile, segment-sum is one TensorE matmul:
      psum_aggT[128 f', 128 n] += G2[e,f']^T-form @ S^T[e,n]
  where S^T[e,n] = (dst_local[e] == n) is built by one VectorE is_equal
  against an iota row, and G2 is the gathered tile.
* z^T = W2^T @ aggT with W2 = [W; W] stacked (folds the hi+lo sum into the
  linear layer).  The bias b cancels in BatchNorm and is dropped.
* BN stats: ScalarE activation accum_out gives per-feature sums of z and z^2;
  a 1KB AllReduce combines the 8 cores; normalization is a per-partition
  (feature) tensor_scalar over z^T.  Output is written as y^T [128, 12544]
  per core; the host transposes and concatenates.
"""

import numpy as np
import ml_dtypes
from contextlib import ExitStack

import concourse.bass as bass
import concourse.tile as tile
import concourse.mybir as mybir
from concourse import bacc
from concourse.bass_utils import run_bass_kernel_spmd


def _install_ntff_hook():
    """Provide antenv.axon_hooks (absent on this image) so trace=True works.

    bass_utils reads the NTFF-profile hook via antenv.axon_hooks; the boot
    module has the ctypes implementation but degrades silently when the
    registry module is missing.  Recreate the registry and register the hook.
    """
    import sys
    import types

    if "antenv.axon_hooks" in sys.modules:
        return
    mod = types.ModuleType("antenv.axon_hooks")
    holder = [None]
    mod.set_axon_ntff_profile_hook = lambda h: holder.__setitem__(0, h)
    mod.get_axon_ntff_profile_hook = lambda: holder[0]
    sys.modules["antenv.axon_hooks"] = mod
    try:
        from trn_agent_boot.trn_boot import _ntff_profile_via_ctypes

        hook = _ntff_profile_via_ctypes("/opt/axon/libaxon_pjrt.so")
        if hook is not None:
            mod.set_axon_ntff_profile_hook(hook)
    except Exception:
        pass


_install_ntff_hook()

BF16 = ml_dtypes.bfloat16

N_NODES = 100000
N_EDGES = 1600000
IN_DIM = 64
HID_DIM = 128
BN_EPS = 1e-5

CORES = 8
NPC = N_NODES // CORES            # 12500 nodes per core
CHUNKS = (NPC + 127) // 128       # 98 chunks of 128 nodes
NCOLS_OUT = CHUNKS * 128          # 12544 (padded output cols per core)
NCLS = 4                          # src index classes (int16 gather limit)
CLS_SZ = N_NODES // NCLS          # 25000
BATCH = 4                         # chunks per gather batch

_compiled = {}                    # Tcg bytes -> (nc, struct)


def _host_prep(h, src, dst, W, gamma, beta):
    """Sort/bucket edges, build per-core gather indices and metadata."""
    h = np.ascontiguousarray(np.asarray(h, dtype=np.float32))
    src = np.asarray(src, dtype=np.int64)
    dst = np.asarray(dst, dtype=np.int64)

    hi = h.astype(BF16)
    lo = (h - hi.astype(np.float32)).astype(BF16)
    h2 = np.concatenate([hi, lo], axis=1)            # [N, 128] bf16

    core = dst // NPC
    dloc = dst - core * NPC
    chunk = dloc >> 7
    dst_local = (dloc & 127).astype(np.float32)
    cls = src // CLS_SZ
    idx16 = (src - cls * CLS_SZ).astype(np.int16)

    key = (core * NCLS + cls) * CHUNKS + chunk
    order = np.argsort(key, kind="stable")
    sk = key[order]

    cnt = np.bincount(key, minlength=CORES * NCLS * CHUNKS)
    cnt = cnt.reshape(CORES, NCLS, CHUNKS)
    Tcg = np.maximum(1, -(-cnt.max(axis=0) // 128)).astype(np.int64)  # [NCLS, CHUNKS]

    # class-stream slot offsets (per class, per chunk), in positions
    off_stream = np.zeros((NCLS, CHUNKS), dtype=np.int64)
    off_stream[:, 1:] = np.cumsum(Tcg[:, :-1], axis=1) * 128
    L = Tcg.sum(axis=1) * 128                                   # [NCLS]

    # dstl column layout: chunk-major, then class, then tile
    n_j = Tcg.sum(axis=0)                                       # [CHUNKS]
    Dbase = np.zeros(CHUNKS, dtype=np.int64)
    Dbase[1:] = np.cumsum(n_j[:-1])
    wco = np.zeros((NCLS, CHUNKS), dtype=np.int64)              # within-chunk col off
    wco[1:] = np.cumsum(Tcg[:-1], axis=0)
    Dtot = int(n_j.sum())

    # rank of each edge within its (core, cls, chunk) bucket (in sorted order)
    grp_starts = np.r_[0, np.flatnonzero(np.diff(sk)) + 1]
    grp_sizes = np.diff(np.r_[grp_starts, len(sk)])
    rank = np.arange(len(sk)) - np.repeat(grp_starts, grp_sizes)

    e_core = core[order]
    e_cls = cls[order]
    e_chunk = chunk[order]
    pos = off_stream[e_cls, e_chunk] + rank                     # class-stream position
    t_edge = (pos >> 7) - (off_stream[e_cls, e_chunk] >> 7)
    p_edge = pos & 127
    col_edge = Dbase[e_chunk] + wco[e_cls, e_chunk] + t_edge

    # per-core, per-class gather index arrays (padded slots gather row 0)
    idx_w = []
    for g in range(NCLS):
        arr = np.zeros((CORES, L[g]), dtype=np.int16)
        m = e_cls == g
        arr[e_core[m], pos[m]] = idx16[order][m]
        # wrap: position i -> [i % 16, i // 16], replicated to 128 partitions
        w = arr.reshape(CORES, L[g] // 16, 16).transpose(0, 2, 1)
        idx_w.append(np.ascontiguousarray(np.tile(w, (1, 8, 1))))

    dstl = np.full((CORES, 128, Dtot), -1.0, dtype=BF16)
    dstl[e_core, p_edge, col_edge] = dst_local[order].astype(BF16)

    W2 = np.concatenate([np.asarray(W, np.float32)] * 2, axis=0)  # [128, 128]
    iota = np.ascontiguousarray(
        np.broadcast_to(np.arange(128, dtype=np.float32).astype(BF16), (128, 128))
    )
    g128 = np.asarray(gamma, np.float32).reshape(HID_DIM, 1)
    b128 = np.asarray(beta, np.float32).reshape(HID_DIM, 1)

    in_maps = []
    for k in range(CORES):
        m = {
            "h2": h2,
            "dstl": np.ascontiguousarray(dstl[k]),
            "w2": W2,
            "gammap": g128,
            "betap": b128,
            "iotap": iota,
        }
        for g in range(NCLS):
            m[f"idx{g}"] = idx_w[g][k]
        in_maps.append(m)

    struct = dict(
        Tcg=Tcg, off_stream=off_stream, L=L, n_j=n_j, Dbase=Dbase, wco=wco,
        Dtot=Dtot,
    )
    return in_maps, struct


def _build(struct, n_cores=CORES, use_collective=True, skip_gather=False,
           skip_compute=False, single_packet=False, max_idx_per_call=8192):
    Tcg = struct["Tcg"]
    off_stream = struct["off_stream"]
    L = struct["L"]
    n_j = struct["n_j"]
    Dbase = struct["Dbase"]
    wco = struct["wco"]
    Dtot = struct["Dtot"]

    f32 = mybir.dt.float32
    bf16 = mybir.dt.bfloat16
    i16 = mybir.dt.int16
    AF = mybir.ActivationFunctionType
    OP = mybir.AluOpType

    nc = bacc.Bacc("TRN2", debug=False)

    h2_t = nc.dram_tensor("h2", [N_NODES, 128], bf16, kind="ExternalInput")
    idx_t = [
        nc.dram_tensor(f"idx{g}", [128, int(L[g]) // 16], i16, kind="ExternalInput")
        for g in range(NCLS)
    ]
    dstl_t = nc.dram_tensor("dstl", [128, Dtot], bf16, kind="ExternalInput")
    w2_t = nc.dram_tensor("w2", [128, 128], f32, kind="ExternalInput")
    gamma_t = nc.dram_tensor("gammap", [128, 1], f32, kind="ExternalInput")
    beta_t = nc.dram_tensor("betap", [128, 1], f32, kind="ExternalInput")
    iota_t = nc.dram_tensor("iotap", [128, 128], bf16, kind="ExternalInput")
    yt_t = nc.dram_tensor("yt", [128, NCOLS_OUT], f32, kind="ExternalOutput")

    nj_max = int(n_j.max())
    # max gather cols for a (batch, class)
    gmax = 0
    for b0 in range(0, CHUNKS, BATCH):
        cs = range(b0, min(CHUNKS, b0 + BATCH))
        for g in range(NCLS):
            gmax = max(gmax, int(sum(Tcg[g, c] for c in cs)))

    with tile.TileContext(nc) as tc, ExitStack() as ctx:
        const = ctx.enter_context(tc.tile_pool(name="const", bufs=1))
        zpool = ctx.enter_context(tc.tile_pool(name="zpool", bufs=1))
        gpools = [
            ctx.enter_context(tc.tile_pool(name=f"gp{g}", bufs=2)) for g in range(NCLS)
        ]
        spool = ctx.enter_context(tc.tile_pool(name="spool", bufs=3))
        apool = ctx.enter_context(tc.tile_pool(name="apool", bufs=2))
        sqpool = ctx.enter_context(tc.tile_pool(name="sqpool", bufs=2))
        stat = ctx.enter_context(tc.tile_pool(name="stat", bufs=1))
        pa_pool = ctx.enter_context(tc.tile_pool(name="pa", bufs=2, space="PSUM"))
        pz_pool = ctx.enter_context(tc.tile_pool(name="pz", bufs=2, space="PSUM"))
        dram = ctx.enter_context(tc.tile_pool(name="dram", bufs=1, space="DRAM"))

        iota_sb = const.tile([128, 128], bf16)
        nc.sync.dma_start(iota_sb[:], iota_t[:])
        w2_sb = const.tile([128, 128], f32)
        nc.sync.dma_start(w2_sb[:], w2_t[:])
        gamma_sb = const.tile([128, 1], f32)
        nc.sync.dma_start(gamma_sb[:], gamma_t[:])
        beta_sb = const.tile([128, 1], f32)
        nc.sync.dma_start(beta_sb[:], beta_t[:])
        dstl_sb = const.tile([128, Dtot], bf16)
        nc.sync.dma_start(dstl_sb[:], dstl_t[:])
        idx_sb = []
        for g in range(NCLS):
            t = const.tile([128, int(L[g]) // 16], i16, name=f"idxsb{g}")
            nc.sync.dma_start(t[:], idx_t[g][:])
            idx_sb.append(t)

        zt = zpool.tile([128, NCOLS_OUT], f32)
        s1c = stat.tile([128, CHUNKS], f32)
        s2c = stat.tile([128, CHUNKS], f32)
        if skip_compute:
            nc.vector.memset(zt[:], 0.0)
            nc.vector.memset(s1c[:], 0.0)
            nc.vector.memset(s2c[:], 1.0)

        for b0 in range(0, CHUNKS, BATCH):
            cs = list(range(b0, min(CHUNKS, b0 + BATCH)))
            gts = []
            for g in range(NCLS):
                cols = int(sum(Tcg[g, c] for c in cs))
                gt = gpools[g].tile([128, gmax, 128], bf16, name=f"gt{g}")
                pos0 = int(off_stream[g, cs[0]])
                npos = cols * 128
                if skip_gather:
                    nc.vector.memset(gt[:, :cols, :], 0.0)
                else:
                    max_cols = max_idx_per_call // 128
                    for c0 in range(0, cols, max_cols):
                        c1 = min(cols, c0 + max_cols)
                        sub = (c1 - c0) * 128
                        p0 = pos0 + c0 * 128
                        nc.gpsimd.dma_gather(
                            gt[:, c0:c1, :],
                            h2_t[g * CLS_SZ : (g + 1) * CLS_SZ, :],
                            idx_sb[g][:, p0 // 16 : (p0 + sub) // 16],
                            sub,
                            sub,
                            128,
                            single_packet=single_packet,
                        )
                gts.append(gt)

            if skip_compute:
                continue
            for c in cs:
                nj = int(n_j[c])
                s_t = spool.tile([128, nj_max, 128], bf16, name="s_t")
                d_sl = dstl_sb[:, int(Dbase[c]) : int(Dbase[c]) + nj]
                nc.vector.tensor_tensor(
                    s_t[:, :nj, :],
                    iota_sb[:].unsqueeze(1).broadcast_to([128, nj, 128]),
                    d_sl.unsqueeze(2).broadcast_to([128, nj, 128]),
                    OP.is_equal,
                )
                pa = pa_pool.tile([128, 128], f32, name="pa")
                j = 0
                for g in range(NCLS):
                    base_col = int(off_stream[g, c] // 128 - off_stream[g, cs[0]] // 128)
                    for t_i in range(int(Tcg[g, c])):
                        nc.tensor.matmul(
                            pa[:],
                            gts[g][:, base_col + t_i, :],
                            s_t[:, j, :],
                            start=(j == 0),
                            stop=(j == nj - 1),
                        )
                        j += 1
                agg_sb = apool.tile([128, 128], f32, name="agg_sb")
                nc.scalar.copy(agg_sb[:], pa[:])
                pz = pz_pool.tile([128, 128], f32, name="pz")
                nc.tensor.matmul(pz[:], w2_sb[:], agg_sb[:], start=True, stop=True)
                nc.scalar.activation(
                    zt[:, c * 128 : (c + 1) * 128], pz[:], AF.Copy,
                    accum_out=s1c[:, c : c + 1],
                )
                sq = sqpool.tile([128, 128], f32, name="sq")
                nc.scalar.activation(
                    sq[:], pz[:], AF.Square, accum_out=s2c[:, c : c + 1],
                )

        # global BN stats
        ccin_sb = stat.tile([128, 2], f32)
        nc.vector.tensor_reduce(ccin_sb[:, 0:1], s1c[:], mybir.AxisListType.X, OP.add)
        nc.vector.tensor_reduce(ccin_sb[:, 1:2], s2c[:], mybir.AxisListType.X, OP.add)
        ccsb = stat.tile([128, 2], f32)
        if n_cores > 1 and use_collective:
            cc_in = dram.tile([128, 2], f32)
            cc_out = dram.tile([128, 2], f32)
            nc.gpsimd.dma_start(cc_in[:], ccin_sb[:])
            nc.gpsimd.collective_compute(
                "AllReduce",
                OP.add,
                replica_groups=[list(range(n_cores))],
                ins=[cc_in.opt()],
                outs=[cc_out.opt()],
            )
            nc.gpsimd.dma_start(ccsb[:], cc_out[:])
        else:
            nc.vector.tensor_copy(ccsb[:], ccin_sb[:])

        n_total = float(N_NODES) if (n_cores > 1 and use_collective) else float(NPC)
        mean = stat.tile([128, 1], f32)
        nc.vector.tensor_scalar_mul(mean[:], ccsb[:, 0:1], 1.0 / n_total)
        ex2 = stat.tile([128, 1], f32)
        nc.vector.tensor_scalar_mul(ex2[:], ccsb[:, 1:2], 1.0 / n_total)
        msq = stat.tile([128, 1], f32)
        nc.vector.tensor_tensor(msq[:], mean[:], mean[:], OP.mult)
        varep = stat.tile([128, 1], f32)
        nc.vector.tensor_tensor(varep[:], ex2[:], msq[:], OP.subtract)
        nc.vector.tensor_scalar_add(varep[:], varep[:], BN_EPS)
        rvar = stat.tile([128, 1], f32)
        nc.vector.reciprocal(rvar[:], varep[:])
        rstd = stat.tile([128, 1], f32)
        nc.scalar.sqrt(rstd[:], rvar[:])
        a_sc = stat.tile([128, 1], f32)
        nc.vector.tensor_tensor(a_sc[:], gamma_sb[:], rstd[:], OP.mult)
        ma = stat.tile([128, 1], f32)
        nc.vector.tensor_tensor(ma[:], mean[:], a_sc[:], OP.mult)
        c_sc = stat.tile([128, 1], f32)
        nc.vector.tensor_tensor(c_sc[:], beta_sb[:], ma[:], OP.subtract)

        nc.vector.tensor_scalar(
            zt[:], zt[:], a_sc[:, 0:1], c_sc[:, 0:1], OP.mult, OP.add
        )
        nc.sync.dma_start(yt_t[:], zt[:])

    nc.compile()
    return nc


def build_and_run(inputs, trace=False, n_cores=CORES):
    in_maps, struct = _host_prep(
        inputs["h"], inputs["src"], inputs["dst"], inputs["W"],
        inputs["gamma"], inputs["beta"],
    )
    key = (struct["Tcg"].tobytes(), n_cores)
    if key not in _compiled:
        _compiled[key] = _build(struct, n_cores=n_cores)
    nc = _compiled[key]
    res = run_bass_kernel_spmd(
        nc, in_maps[:n_cores], core_ids=list(range(n_cores)), trace=trace
    )
    y = np.concatenate(
        [res.results[k]["yt"][:, :NPC].T for k in range(n_cores)], axis=0
    )
    return np.ascontiguousarray(y), res


def kernel(h, src, dst, W, b, gamma, beta):
    y, _ = build_and_run(
        dict(h=h, src=src, dst=dst, W=W, b=b, gamma=gamma, beta=beta)
    )
    return y


# revision 14
# speedup vs baseline: 2.6721x; 2.6721x over previous
"""GCN layer (message passing + linear + BatchNorm) on 8 Trainium2 NeuronCores.

Strategy
--------
* Nodes are sharded contiguously across the 8 cores (12500 nodes each, 98
  chunks of 128 nodes).  Edges are partitioned by dst core so segment_sum is
  local; h is replicated (gathered directly from each core's HBM copy).
* h is pre-split on host into bf16 (hi, lo) pairs packed as h2[N,128] bf16 so
  one gathered 256B row reconstructs the fp32 value exactly
  (hi + lo == fp32 h to ~2^-17 relative).
* dma_gather uses int16 indices (max 32767), so src ids are classed by range
  (4 classes of 25000 rows); edges are bucketed (class, chunk), padded to
  128-edge tiles.
* Per 128-edge tile, segment-sum is one TensorE matmul:
      psum_aggT[128 f', 128 n] += G2[e,f']^T-form @ S^T[e,n]
  where S^T[e,n] = (dst_local[e] == n) is built by one VectorE is_equal
  against an iota row, and G2 is the gathered tile.
* z^T = W2^T @ aggT with W2 = [W; W] stacked (folds the hi+lo sum into the
  linear layer).  The bias b cancels in BatchNorm and is dropped.
* BN stats: ScalarE activation accum_out gives per-feature sums of z and z^2;
  a 1KB AllReduce combines the 8 cores; normalization is a per-partition
  (feature) tensor_scalar over z^T.  Output is written as y^T [128, 12544]
  per core; the host transposes and concatenates.
"""

import numpy as np
import ml_dtypes
from contextlib import ExitStack

import concourse.bass as bass
import concourse.tile as tile
import concourse.mybir as mybir
from concourse import bacc
from concourse.bass_utils import run_bass_kernel_spmd


def _install_ntff_hook():
    """Provide antenv.axon_hooks (absent on this image) so trace=True works.

    bass_utils reads the NTFF-profile hook via antenv.axon_hooks; the boot
    module has the ctypes implementation but degrades silently when the
    registry module is missing.  Recreate the registry and register the hook.
    """
    import sys
    import types

    if "antenv.axon_hooks" in sys.modules:
        return
    mod = types.ModuleType("antenv.axon_hooks")
    holder = [None]
    mod.set_axon_ntff_profile_hook = lambda h: holder.__setitem__(0, h)
    mod.get_axon_ntff_profile_hook = lambda: holder[0]
    sys.modules["antenv.axon_hooks"] = mod
    try:
        from trn_agent_boot.trn_boot import _ntff_profile_via_ctypes

        hook = _ntff_profile_via_ctypes("/opt/axon/libaxon_pjrt.so")
        if hook is not None:
            mod.set_axon_ntff_profile_hook(hook)
    except Exception:
        pass


_install_ntff_hook()

BF16 = ml_dtypes.bfloat16

N_NODES = 100000
N_EDGES = 1600000
IN_DIM = 64
HID_DIM = 128
BN_EPS = 1e-5

CORES = 8
NPC = N_NODES // CORES            # 12500 nodes per core
CHUNKS = (NPC + 127) // 128       # 98 chunks of 128 nodes
NCOLS_OUT = CHUNKS * 128          # 12544 (padded output cols per core)
NCLS = 4                          # src index classes (int16 gather limit)
CLS_SZ = N_NODES // NCLS          # 25000
BATCH = 4                         # chunks per gather batch

_compiled = {}                    # Tcg bytes -> (nc, struct)


def _host_prep(h, src, dst, W, gamma, beta):
    """Sort/bucket edges, build per-core gather indices and metadata."""
    h = np.ascontiguousarray(np.asarray(h, dtype=np.float32))
    src = np.asarray(src, dtype=np.int64)
    dst = np.asarray(dst, dtype=np.int64)

    hi = h.astype(BF16)
    lo = (h - hi.astype(np.float32)).astype(BF16)
    h2 = np.concatenate([hi, lo], axis=1)            # [N, 128] bf16

    core = dst // NPC
    dloc = dst - core * NPC
    chunk = dloc >> 7
    dst_local = (dloc & 127).astype(np.float32)
    cls = src // CLS_SZ
    idx16 = (src - cls * CLS_SZ).astype(np.int16)

    key = (core * NCLS + cls) * CHUNKS + chunk
    order = np.argsort(key, kind="stable")
    sk = key[order]

    cnt = np.bincount(key, minlength=CORES * NCLS * CHUNKS)
    cnt = cnt.reshape(CORES, NCLS, CHUNKS)
    Tcg = np.maximum(1, -(-cnt.max(axis=0) // 128)).astype(np.int64)  # [NCLS, CHUNKS]

    # class-stream slot offsets (per class, per chunk), in positions
    off_stream = np.zeros((NCLS, CHUNKS), dtype=np.int64)
    off_stream[:, 1:] = np.cumsum(Tcg[:, :-1], axis=1) * 128
    L = Tcg.sum(axis=1) * 128                                   # [NCLS]

    # dstl column layout: chunk-major, then class, then tile
    n_j = Tcg.sum(axis=0)                                       # [CHUNKS]
    Dbase = np.zeros(CHUNKS, dtype=np.int64)
    Dbase[1:] = np.cumsum(n_j[:-1])
    wco = np.zeros((NCLS, CHUNKS), dtype=np.int64)              # within-chunk col off
    wco[1:] = np.cumsum(Tcg[:-1], axis=0)
    Dtot = int(n_j.sum())

    # rank of each edge within its (core, cls, chunk) bucket (in sorted order)
    grp_starts = np.r_[0, np.flatnonzero(np.diff(sk)) + 1]
    grp_sizes = np.diff(np.r_[grp_starts, len(sk)])
    rank = np.arange(len(sk)) - np.repeat(grp_starts, grp_sizes)

    e_core = core[order]
    e_cls = cls[order]
    e_chunk = chunk[order]
    pos = off_stream[e_cls, e_chunk] + rank                     # class-stream position
    t_edge = (pos >> 7) - (off_stream[e_cls, e_chunk] >> 7)
    p_edge = pos & 127
    col_edge = Dbase[e_chunk] + wco[e_cls, e_chunk] + t_edge

    # per-core, per-class gather index arrays (padded slots gather row 0)
    idx_w = []
    for g in range(NCLS):
        arr = np.zeros((CORES, L[g]), dtype=np.int16)
        m = e_cls == g
        arr[e_core[m], pos[m]] = idx16[order][m]
        # wrap: position i -> [i % 16, i // 16], replicated to 128 partitions
        w = arr.reshape(CORES, L[g] // 16, 16).transpose(0, 2, 1)
        idx_w.append(np.ascontiguousarray(np.tile(w, (1, 8, 1))))

    dstl = np.full((CORES, 128, Dtot), -1.0, dtype=BF16)
    dstl[e_core, p_edge, col_edge] = dst_local[order].astype(BF16)

    W2 = np.concatenate([np.asarray(W, np.float32)] * 2, axis=0)  # [128, 128]
    iota = np.ascontiguousarray(
        np.broadcast_to(np.arange(128, dtype=np.float32).astype(BF16), (128, 128))
    )
    g128 = np.asarray(gamma, np.float32).reshape(HID_DIM, 1)
    b128 = np.asarray(beta, np.float32).reshape(HID_DIM, 1)

    in_maps = []
    for k in range(CORES):
        m = {
            "h2": h2,
            "dstl": np.ascontiguousarray(dstl[k]),
            "w2": W2,
            "gammap": g128,
            "betap": b128,
            "iotap": iota,
        }
        for g in range(NCLS):
            m[f"idx{g}"] = idx_w[g][k]
        in_maps.append(m)

    struct = dict(
        Tcg=Tcg, off_stream=off_stream, L=L, n_j=n_j, Dbase=Dbase, wco=wco,
        Dtot=Dtot,
    )
    return in_maps, struct


def _build(struct, n_cores=CORES, use_collective=True, skip_gather=False,
           skip_compute=False, single_packet=False, max_idx_per_call=8192,
           n_queues=4):
    Tcg = struct["Tcg"]
    off_stream = struct["off_stream"]
    L = struct["L"]
    n_j = struct["n_j"]
    Dbase = struct["Dbase"]
    wco = struct["wco"]
    Dtot = struct["Dtot"]

    f32 = mybir.dt.float32
    bf16 = mybir.dt.bfloat16
    i16 = mybir.dt.int16
    AF = mybir.ActivationFunctionType
    OP = mybir.AluOpType

    nc = bacc.Bacc("TRN2", debug=False, num_swdge_queues=n_queues)

    h2_t = nc.dram_tensor("h2", [N_NODES, 128], bf16, kind="ExternalInput")
    idx_t = [
        nc.dram_tensor(f"idx{g}", [128, int(L[g]) // 16], i16, kind="ExternalInput")
        for g in range(NCLS)
    ]
    dstl_t = nc.dram_tensor("dstl", [128, Dtot], bf16, kind="ExternalInput")
    w2_t = nc.dram_tensor("w2", [128, 128], f32, kind="ExternalInput")
    gamma_t = nc.dram_tensor("gammap", [128, 1], f32, kind="ExternalInput")
    beta_t = nc.dram_tensor("betap", [128, 1], f32, kind="ExternalInput")
    iota_t = nc.dram_tensor("iotap", [128, 128], bf16, kind="ExternalInput")
    yt_t = nc.dram_tensor("yt", [128, NCOLS_OUT], f32, kind="ExternalOutput")

    nj_max = int(n_j.max())
    # max gather cols for a (batch, class)
    gmax = 0
    for b0 in range(0, CHUNKS, BATCH):
        cs = range(b0, min(CHUNKS, b0 + BATCH))
        for g in range(NCLS):
            gmax = max(gmax, int(sum(Tcg[g, c] for c in cs)))

    with tile.TileContext(nc) as tc, ExitStack() as ctx:
        const = ctx.enter_context(tc.tile_pool(name="const", bufs=1))
        zpool = ctx.enter_context(tc.tile_pool(name="zpool", bufs=1))
        gpools = [
            ctx.enter_context(tc.tile_pool(name=f"gp{g}", bufs=2)) for g in range(NCLS)
        ]
        spool = ctx.enter_context(tc.tile_pool(name="spool", bufs=3))
        apool = ctx.enter_context(tc.tile_pool(name="apool", bufs=2))
        sqpool = ctx.enter_context(tc.tile_pool(name="sqpool", bufs=2))
        stat = ctx.enter_context(tc.tile_pool(name="stat", bufs=1))
        pa_pool = ctx.enter_context(tc.tile_pool(name="pa", bufs=2, space="PSUM"))
        pz_pool = ctx.enter_context(tc.tile_pool(name="pz", bufs=2, space="PSUM"))
        dram = ctx.enter_context(tc.tile_pool(name="dram", bufs=1, space="DRAM"))

        iota_sb = const.tile([128, 128], bf16)
        nc.sync.dma_start(iota_sb[:], iota_t[:])
        w2_sb = const.tile([128, 128], f32)
        nc.sync.dma_start(w2_sb[:], w2_t[:])
        gamma_sb = const.tile([128, 1], f32)
        nc.sync.dma_start(gamma_sb[:], gamma_t[:])
        beta_sb = const.tile([128, 1], f32)
        nc.sync.dma_start(beta_sb[:], beta_t[:])
        dstl_sb = const.tile([128, Dtot], bf16)
        nc.sync.dma_start(dstl_sb[:], dstl_t[:])
        idx_sb = []
        for g in range(NCLS):
            t = const.tile([128, int(L[g]) // 16], i16, name=f"idxsb{g}")
            nc.sync.dma_start(t[:], idx_t[g][:])
            idx_sb.append(t)

        zt = zpool.tile([128, NCOLS_OUT], f32)
        s1c = stat.tile([128, CHUNKS], f32)
        s2c = stat.tile([128, CHUNKS], f32)
        if skip_compute:
            nc.vector.memset(zt[:], 0.0)
            nc.vector.memset(s1c[:], 0.0)
            nc.vector.memset(s2c[:], 1.0)

        for b0 in range(0, CHUNKS, BATCH):
            cs = list(range(b0, min(CHUNKS, b0 + BATCH)))
            gts = []
            for g in range(NCLS):
                cols = int(sum(Tcg[g, c] for c in cs))
                gt = gpools[g].tile([128, gmax, 128], bf16, name=f"gt{g}")
                pos0 = int(off_stream[g, cs[0]])
                npos = cols * 128
                if skip_gather:
                    nc.vector.memset(gt[:, :cols, :], 0.0)
                else:
                    max_cols = max_idx_per_call // 128
                    for c0 in range(0, cols, max_cols):
                        c1 = min(cols, c0 + max_cols)
                        sub = (c1 - c0) * 128
                        p0 = pos0 + c0 * 128
                        nc.gpsimd.dma_gather(
                            gt[:, c0:c1, :],
                            h2_t[g * CLS_SZ : (g + 1) * CLS_SZ, :],
                            idx_sb[g][:, p0 // 16 : (p0 + sub) // 16],
                            sub,
                            sub,
                            128,
                            single_packet=single_packet,
                            queue_num=g % n_queues,
                        )
                gts.append(gt)

            if skip_compute:
                continue
            for c in cs:
                nj = int(n_j[c])
                s_t = spool.tile([128, nj_max, 128], bf16, name="s_t")
                d_sl = dstl_sb[:, int(Dbase[c]) : int(Dbase[c]) + nj]
                nc.vector.tensor_tensor(
                    s_t[:, :nj, :],
                    iota_sb[:].unsqueeze(1).broadcast_to([128, nj, 128]),
                    d_sl.unsqueeze(2).broadcast_to([128, nj, 128]),
                    OP.is_equal,
                )
                pa = pa_pool.tile([128, 128], f32, name="pa")
                j = 0
                for g in range(NCLS):
                    base_col = int(off_stream[g, c] // 128 - off_stream[g, cs[0]] // 128)
                    for t_i in range(int(Tcg[g, c])):
                        nc.tensor.matmul(
                            pa[:],
                            gts[g][:, base_col + t_i, :],
                            s_t[:, j, :],
                            start=(j == 0),
                            stop=(j == nj - 1),
                        )
                        j += 1
                agg_sb = apool.tile([128, 128], f32, name="agg_sb")
                nc.scalar.copy(agg_sb[:], pa[:])
                pz = pz_pool.tile([128, 128], f32, name="pz")
                nc.tensor.matmul(pz[:], w2_sb[:], agg_sb[:], start=True, stop=True)
                nc.scalar.activation(
                    zt[:, c * 128 : (c + 1) * 128], pz[:], AF.Copy,
                    accum_out=s1c[:, c : c + 1],
                )
                sq = sqpool.tile([128, 128], f32, name="sq")
                nc.scalar.activation(
                    sq[:], pz[:], AF.Square, accum_out=s2c[:, c : c + 1],
                )

        # global BN stats
        ccin_sb = stat.tile([128, 2], f32)
        nc.vector.tensor_reduce(ccin_sb[:, 0:1], s1c[:], mybir.AxisListType.X, OP.add)
        nc.vector.tensor_reduce(ccin_sb[:, 1:2], s2c[:], mybir.AxisListType.X, OP.add)
        ccsb = stat.tile([128, 2], f32)
        if n_cores > 1 and use_collective:
            cc_in = dram.tile([128, 2], f32)
            cc_out = dram.tile([128, 2], f32)
            nc.gpsimd.dma_start(cc_in[:], ccin_sb[:])
            nc.gpsimd.collective_compute(
                "AllReduce",
                OP.add,
                replica_groups=[list(range(n_cores))],
                ins=[cc_in.opt()],
                outs=[cc_out.opt()],
            )
            nc.gpsimd.dma_start(ccsb[:], cc_out[:])
        else:
            nc.vector.tensor_copy(ccsb[:], ccin_sb[:])

        n_total = float(N_NODES) if (n_cores > 1 and use_collective) else float(NPC)
        mean = stat.tile([128, 1], f32)
        nc.vector.tensor_scalar_mul(mean[:], ccsb[:, 0:1], 1.0 / n_total)
        ex2 = stat.tile([128, 1], f32)
        nc.vector.tensor_scalar_mul(ex2[:], ccsb[:, 1:2], 1.0 / n_total)
        msq = stat.tile([128, 1], f32)
        nc.vector.tensor_tensor(msq[:], mean[:], mean[:], OP.mult)
        varep = stat.tile([128, 1], f32)
        nc.vector.tensor_tensor(varep[:], ex2[:], msq[:], OP.subtract)
        nc.vector.tensor_scalar_add(varep[:], varep[:], BN_EPS)
        rvar = stat.tile([128, 1], f32)
        nc.vector.reciprocal(rvar[:], varep[:])
        rstd = stat.tile([128, 1], f32)
        nc.scalar.sqrt(rstd[:], rvar[:])
        a_sc = stat.tile([128, 1], f32)
        nc.vector.tensor_tensor(a_sc[:], gamma_sb[:], rstd[:], OP.mult)
        ma = stat.tile([128, 1], f32)
        nc.vector.tensor_tensor(ma[:], mean[:], a_sc[:], OP.mult)
        c_sc = stat.tile([128, 1], f32)
        nc.vector.tensor_tensor(c_sc[:], beta_sb[:], ma[:], OP.subtract)

        nc.vector.tensor_scalar(
            zt[:], zt[:], a_sc[:, 0:1], c_sc[:, 0:1], OP.mult, OP.add
        )
        nc.sync.dma_start(yt_t[:], zt[:])

    nc.compile()
    return nc


def build_and_run(inputs, trace=False, n_cores=CORES):
    in_maps, struct = _host_prep(
        inputs["h"], inputs["src"], inputs["dst"], inputs["W"],
        inputs["gamma"], inputs["beta"],
    )
    key = (struct["Tcg"].tobytes(), n_cores)
    if key not in _compiled:
        _compiled[key] = _build(struct, n_cores=n_cores)
    nc = _compiled[key]
    res = run_bass_kernel_spmd(
        nc, in_maps[:n_cores], core_ids=list(range(n_cores)), trace=trace
    )
    y = np.concatenate(
        [res.results[k]["yt"][:, :NPC].T for k in range(n_cores)], axis=0
    )
    return np.ascontiguousarray(y), res


def kernel(h, src, dst, W, b, gamma, beta):
    y, _ = build_and_run(
        dict(h=h, src=src, dst=dst, W=W, b=b, gamma=gamma, beta=beta)
    )
    return y


# revision 17
# speedup vs baseline: 3.3287x; 1.2457x over previous
"""GCN layer (message passing + linear + BatchNorm) on 8 Trainium2 NeuronCores.

Strategy
--------
* Nodes are sharded across the 8 cores (12500 each) and bin-packed per core
  into C chunks of <=128 nodes such that each (chunk, src-class) edge bucket
  holds <= 512 edges (exactly 4 x 128-edge tiles).  Edges are partitioned by
  dst core so segment_sum is local; h is replicated.
* h is pre-split on host into bf16 (hi, lo) pairs packed as h2[N,128] bf16 so
  one gathered 256B row reconstructs the fp32 value exactly.
* dma_gather uses int16 indices (max 32767), so src ids are classed by range
  (4 classes of 25000 rows).  One gather call per (4-chunk batch, class) on
  its own SWDGE queue (4 queues -> 4x parallel Q7 descriptor generation).
* Per 128-edge tile, segment-sum is one TensorE matmul:
      psum_aggT[128 f', 128 n] += G2-as-lhsT @ S^T[e,n]
  where S^T[e,n] = (dst_local[e] == n) is built by VectorE is_equal against
  an iota row, and G2 is the gathered tile.
* z^T = W2^T @ aggT with W2 = [W; W] stacked (folds the hi+lo sum into the
  linear layer).  The bias b cancels in BatchNorm and is dropped.
* BN stats: ScalarE activation accum_out gives per-feature sums of z and z^2;
  a 1KB AllReduce combines the 8 cores; normalization is a per-partition
  (feature) tensor_scalar over z^T.  Output is written as y^T [128, C*128]
  per core; the host inverse-permutes and concatenates.
"""

import numpy as np
import ml_dtypes
from contextlib import ExitStack

import concourse.bass as bass
import concourse.tile as tile
import concourse.mybir as mybir
from concourse import bacc
from concourse.bass_utils import run_bass_kernel_spmd


def _install_ntff_hook():
    """Provide antenv.axon_hooks (absent on this image) so trace=True works."""
    import sys
    import types

    if "antenv.axon_hooks" in sys.modules:
        return
    mod = types.ModuleType("antenv.axon_hooks")
    holder = [None]
    mod.set_axon_ntff_profile_hook = lambda h: holder.__setitem__(0, h)
    mod.get_axon_ntff_profile_hook = lambda: holder[0]
    sys.modules["antenv.axon_hooks"] = mod
    try:
        from trn_agent_boot.trn_boot import _ntff_profile_via_ctypes

        hook = _ntff_profile_via_ctypes("/opt/axon/libaxon_pjrt.so")
        if hook is not None:
            mod.set_axon_ntff_profile_hook(hook)
    except Exception:
        pass


_install_ntff_hook()

BF16 = ml_dtypes.bfloat16

N_NODES = 100000
N_EDGES = 1600000
IN_DIM = 64
HID_DIM = 128
BN_EPS = 1e-5

CORES = 8
NPC = N_NODES // CORES            # 12500 nodes per core
NCLS = 4                          # src index classes (int16 gather limit)
CLS_SZ = N_NODES // NCLS          # 25000
TPB = 4                           # tiles (of 128 edges) per (chunk, class)
BUCKET_CAP = TPB * 128            # 512 edges per (chunk, class)
BATCH = 4                         # chunks per gather batch

_compiled = {}


def _pack_chunks(deg4, node_cap=128):
    """Next-fit pack nodes into chunks with per-class edge cap and node cap.

    deg4: [n_nodes, 4] per-class in-degree.  Returns (chunk_id, slot_id, C).
    """
    n = deg4.shape[0]
    chunk_id = np.zeros(n, np.int64)
    slot_id = np.zeros(n, np.int64)
    cur = 0
    fill = [0, 0, 0, 0]
    nodes_in = 0
    for i in range(n):
        d = deg4[i]
        if (
            nodes_in >= node_cap
            or fill[0] + d[0] > BUCKET_CAP
            or fill[1] + d[1] > BUCKET_CAP
            or fill[2] + d[2] > BUCKET_CAP
            or fill[3] + d[3] > BUCKET_CAP
        ):
            cur += 1
            fill = [0, 0, 0, 0]
            nodes_in = 0
        chunk_id[i] = cur
        slot_id[i] = nodes_in
        fill[0] += d[0]
        fill[1] += d[1]
        fill[2] += d[2]
        fill[3] += d[3]
        nodes_in += 1
    return chunk_id, slot_id, cur + 1


def _host_prep(h, src, dst, W, gamma, beta):
    h = np.ascontiguousarray(np.asarray(h, dtype=np.float32))
    src = np.asarray(src, dtype=np.int64)
    dst = np.asarray(dst, dtype=np.int64)

    hi = h.astype(BF16)
    lo = (h - hi.astype(np.float32)).astype(BF16)
    h2 = np.concatenate([hi, lo], axis=1)            # [N, 128] bf16

    core = dst // NPC
    cls = src // CLS_SZ
    idx16 = (src - cls * CLS_SZ).astype(np.int16)

    # per-node per-class in-degree -> per-core chunk packing
    deg4 = np.bincount(dst * NCLS + cls, minlength=N_NODES * NCLS).reshape(
        N_NODES, NCLS
    )
    chunk_all = np.zeros(N_NODES, np.int64)
    slot_all = np.zeros(N_NODES, np.int64)
    Cs = []
    for k in range(CORES):
        sl = slice(k * NPC, (k + 1) * NPC)
        c_id, s_id, Ck = _pack_chunks(deg4[sl])
        chunk_all[sl] = c_id
        slot_all[sl] = s_id
        Cs.append(Ck)
    C = int(max(Cs))

    chunk = chunk_all[dst]
    dst_local = slot_all[dst].astype(np.float32)

    # rank of each edge within its (core, cls, chunk) bucket
    key = (core * NCLS + cls) * C + chunk
    order = np.argsort(key, kind="stable")
    sk = key[order]
    grp_starts = np.r_[0, np.flatnonzero(np.diff(sk)) + 1]
    grp_sizes = np.diff(np.r_[grp_starts, len(sk)])
    rank = np.arange(len(sk)) - np.repeat(grp_starts, grp_sizes)
    assert rank.max() < BUCKET_CAP

    e_core = core[order]
    e_cls = cls[order]
    e_chunk = chunk[order]
    pos = e_chunk * BUCKET_CAP + rank            # class-stream position
    t_edge = rank >> 7
    p_edge = pos & 127
    col_edge = e_chunk * (NCLS * TPB) + e_cls * TPB + t_edge

    L = C * BUCKET_CAP                           # positions per class stream
    Dtot = C * NCLS * TPB

    idx_w = []
    for g in range(NCLS):
        arr = np.zeros((CORES, L), dtype=np.int16)
        m = e_cls == g
        arr[e_core[m], pos[m]] = idx16[order][m]
        w = arr.reshape(CORES, L // 16, 16).transpose(0, 2, 1)
        idx_w.append(np.ascontiguousarray(np.tile(w, (1, 8, 1))))

    dstl = np.full((CORES, 128, Dtot), -1.0, dtype=BF16)
    dstl[e_core, p_edge, col_edge] = dst_local[order].astype(BF16)

    W2 = np.concatenate([np.asarray(W, np.float32)] * 2, axis=0)  # [128, 128]
    iota = np.ascontiguousarray(
        np.broadcast_to(np.arange(128, dtype=np.float32).astype(BF16), (128, 128))
    )
    g128 = np.asarray(gamma, np.float32).reshape(HID_DIM, 1)
    b128 = np.asarray(beta, np.float32).reshape(HID_DIM, 1)

    in_maps = []
    for k in range(CORES):
        m = {
            "h2": h2,
            "dstl": np.ascontiguousarray(dstl[k]),
            "w2": W2,
            "gammap": g128,
            "betap": b128,
            "iotap": iota,
        }
        for g in range(NCLS):
            m[f"idx{g}"] = idx_w[g][k]
        in_maps.append(m)

    colmap = (chunk_all * 128 + slot_all).reshape(CORES, NPC)
    struct = dict(C=C, L=L, Dtot=Dtot, colmap=colmap)
    return in_maps, struct


def _build(struct, n_cores=CORES, use_collective=True, skip_gather=False,
           skip_compute=False, single_packet=False, n_queues=4):
    C = struct["C"]
    L = struct["L"]
    Dtot = struct["Dtot"]
    ncols_out = C * 128

    f32 = mybir.dt.float32
    bf16 = mybir.dt.bfloat16
    i16 = mybir.dt.int16
    AF = mybir.ActivationFunctionType
    OP = mybir.AluOpType

    nc = bacc.Bacc("TRN2", debug=False, num_swdge_queues=n_queues)

    h2_t = nc.dram_tensor("h2", [N_NODES, 128], bf16, kind="ExternalInput")
    idx_t = [
        nc.dram_tensor(f"idx{g}", [128, L // 16], i16, kind="ExternalInput")
        for g in range(NCLS)
    ]
    dstl_t = nc.dram_tensor("dstl", [128, Dtot], bf16, kind="ExternalInput")
    w2_t = nc.dram_tensor("w2", [128, 128], f32, kind="ExternalInput")
    gamma_t = nc.dram_tensor("gammap", [128, 1], f32, kind="ExternalInput")
    beta_t = nc.dram_tensor("betap", [128, 1], f32, kind="ExternalInput")
    iota_t = nc.dram_tensor("iotap", [128, 128], bf16, kind="ExternalInput")
    yt_t = nc.dram_tensor("yt", [128, ncols_out], f32, kind="ExternalOutput")

    NJ = NCLS * TPB  # sub-tiles (matmuls) per chunk = 16

    with tile.TileContext(nc) as tc, ExitStack() as ctx:
        const = ctx.enter_context(tc.tile_pool(name="const", bufs=1))
        zpool = ctx.enter_context(tc.tile_pool(name="zpool", bufs=1))
        gpools = [
            ctx.enter_context(tc.tile_pool(name=f"gp{g}", bufs=2)) for g in range(NCLS)
        ]
        spool = ctx.enter_context(tc.tile_pool(name="spool", bufs=3))
        apool = ctx.enter_context(tc.tile_pool(name="apool", bufs=2))
        sqpool = ctx.enter_context(tc.tile_pool(name="sqpool", bufs=2))
        stat = ctx.enter_context(tc.tile_pool(name="stat", bufs=1))
        pa_pool = ctx.enter_context(tc.tile_pool(name="pa", bufs=2, space="PSUM"))
        pz_pool = ctx.enter_context(tc.tile_pool(name="pz", bufs=2, space="PSUM"))
        dram = ctx.enter_context(tc.tile_pool(name="dram", bufs=1, space="DRAM"))

        iota_sb = const.tile([128, 128], bf16)
        nc.sync.dma_start(iota_sb[:], iota_t[:])
        w2_sb = const.tile([128, 128], f32)
        nc.sync.dma_start(w2_sb[:], w2_t[:])
        gamma_sb = const.tile([128, 1], f32)
        nc.sync.dma_start(gamma_sb[:], gamma_t[:])
        beta_sb = const.tile([128, 1], f32)
        nc.sync.dma_start(beta_sb[:], beta_t[:])
        dstl_sb = const.tile([128, Dtot], bf16)
        nc.sync.dma_start(dstl_sb[:], dstl_t[:])
        idx_sb = []
        for g in range(NCLS):
            t = const.tile([128, L // 16], i16, name=f"idxsb{g}")
            nc.sync.dma_start(t[:], idx_t[g][:])
            idx_sb.append(t)

        zt = zpool.tile([128, ncols_out], f32)
        s1c = stat.tile([128, C], f32)
        s2c = stat.tile([128, C], f32)
        if skip_compute:
            nc.vector.memset(zt[:], 0.0)
            nc.vector.memset(s1c[:], 0.0)
            nc.vector.memset(s2c[:], 1.0)

        for b0 in range(0, C, BATCH):
            cs = list(range(b0, min(C, b0 + BATCH)))
            nb = len(cs)
            gts = []
            for g in range(NCLS):
                cols = nb * TPB
                gt = gpools[g].tile([128, BATCH * TPB, 128], bf16, name=f"gt{g}")
                pos0 = b0 * BUCKET_CAP
                npos = cols * 128
                if skip_gather:
                    nc.vector.memset(gt[:, :cols, :], 0.0)
                else:
                    nc.gpsimd.dma_gather(
                        gt[:, :cols, :],
                        h2_t[g * CLS_SZ : (g + 1) * CLS_SZ, :],
                        idx_sb[g][:, pos0 // 16 : (pos0 + npos) // 16],
                        npos,
                        npos,
                        128,
                        single_packet=single_packet,
                        queue_num=g % n_queues,
                    )
                gts.append(gt)

            if skip_compute:
                continue
            for c in cs:
                s_t = spool.tile([128, NJ, 128], bf16, name="s_t")
                d_sl = dstl_sb[:, c * NJ : (c + 1) * NJ]
                nc.vector.tensor_tensor(
                    s_t[:],
                    iota_sb[:].unsqueeze(1).broadcast_to([128, NJ, 128]),
                    d_sl.unsqueeze(2).broadcast_to([128, NJ, 128]),
                    OP.is_equal,
                )
                pa = pa_pool.tile([128, 128], f32, name="pa")
                j = 0
                for g in range(NCLS):
                    for t_i in range(TPB):
                        nc.tensor.matmul(
                            pa[:],
                            gts[g][:, (c - b0) * TPB + t_i, :],
                            s_t[:, j, :],
                            start=(j == 0),
                            stop=(j == NJ - 1),
                        )
                        j += 1
                agg_sb = apool.tile([128, 128], f32, name="agg_sb")
                nc.scalar.copy(agg_sb[:], pa[:])
                pz = pz_pool.tile([128, 128], f32, name="pz")
                nc.tensor.matmul(pz[:], w2_sb[:], agg_sb[:], start=True, stop=True)
                nc.scalar.activation(
                    zt[:, c * 128 : (c + 1) * 128], pz[:], AF.Copy,
                    accum_out=s1c[:, c : c + 1],
                )
                sq = sqpool.tile([128, 128], f32, name="sq")
                nc.scalar.activation(
                    sq[:], pz[:], AF.Square, accum_out=s2c[:, c : c + 1],
                )

        # global BN stats
        ccin_sb = stat.tile([128, 2], f32)
        nc.vector.tensor_reduce(ccin_sb[:, 0:1], s1c[:], mybir.AxisListType.X, OP.add)
        nc.vector.tensor_reduce(ccin_sb[:, 1:2], s2c[:], mybir.AxisListType.X, OP.add)
        ccsb = stat.tile([128, 2], f32)
        if n_cores > 1 and use_collective:
            cc_in = dram.tile([128, 2], f32)
            cc_out = dram.tile([128, 2], f32)
            nc.sync.dma_start(cc_in[:], ccin_sb[:])
            nc.gpsimd.collective_compute(
                "AllReduce",
                OP.add,
                replica_groups=[list(range(n_cores))],
                ins=[cc_in.opt()],
                outs=[cc_out.opt()],
            )
            nc.sync.dma_start(ccsb[:], cc_out[:])
        else:
            nc.vector.tensor_copy(ccsb[:], ccin_sb[:])

        n_total = float(N_NODES) if (n_cores > 1 and use_collective) else float(NPC)
        mean = stat.tile([128, 1], f32)
        nc.vector.tensor_scalar_mul(mean[:], ccsb[:, 0:1], 1.0 / n_total)
        ex2 = stat.tile([128, 1], f32)
        nc.vector.tensor_scalar_mul(ex2[:], ccsb[:, 1:2], 1.0 / n_total)
        msq = stat.tile([128, 1], f32)
        nc.vector.tensor_tensor(msq[:], mean[:], mean[:], OP.mult)
        varep = stat.tile([128, 1], f32)
        nc.vector.tensor_tensor(varep[:], ex2[:], msq[:], OP.subtract)
        nc.vector.tensor_scalar_add(varep[:], varep[:], BN_EPS)
        rvar = stat.tile([128, 1], f32)
        nc.vector.reciprocal(rvar[:], varep[:])
        rstd = stat.tile([128, 1], f32)
        nc.scalar.sqrt(rstd[:], rvar[:])
        a_sc = stat.tile([128, 1], f32)
        nc.vector.tensor_tensor(a_sc[:], gamma_sb[:], rstd[:], OP.mult)
        ma = stat.tile([128, 1], f32)
        nc.vector.tensor_tensor(ma[:], mean[:], a_sc[:], OP.mult)
        c_sc = stat.tile([128, 1], f32)
        nc.vector.tensor_tensor(c_sc[:], beta_sb[:], ma[:], OP.subtract)

        # normalize + store in 4 column blocks (overlap DVE with output DMA)
        nblk = (ncols_out + 3) // 4
        for i0 in range(0, ncols_out, nblk):
            i1 = min(ncols_out, i0 + nblk)
            nc.vector.tensor_scalar(
                zt[:, i0:i1], zt[:, i0:i1], a_sc[:, 0:1], c_sc[:, 0:1],
                OP.mult, OP.add,
            )
            nc.sync.dma_start(yt_t[:, i0:i1], zt[:, i0:i1])

    # Tile assigns SWDGE completion-sem lanes (DMASW{i}) round-robin in
    # scheduled order, and each lane is locked to one SWDGE queue at first
    # use.  Rewrite each gather's queue to lane % n_queues so the mapping is
    # consistent (keeps the 4-way parallel Q7 descriptor generation).
    import re

    for blk in nc.m.functions[0].blocks:
        for inst in blk.instructions:
            if isinstance(inst, mybir.InstDMAGatherAnt):
                si = inst.sync_info
                if si is None or not si.on_update:
                    continue
                m = re.match(r"DMASW(\d+)", si.on_update[0].ant_name or "")
                if m:
                    inst.queue_num = int(m.group(1)) % n_queues

    nc.compile()
    return nc


def build_and_run(inputs, trace=False, n_cores=CORES, **build_kw):
    in_maps, struct = _host_prep(
        inputs["h"], inputs["src"], inputs["dst"], inputs["W"],
        inputs["gamma"], inputs["beta"],
    )
    key = (struct["C"], n_cores, tuple(sorted(build_kw.items())))
    if key not in _compiled:
        _compiled[key] = _build(struct, n_cores=n_cores, **build_kw)
    nc = _compiled[key]
    res = run_bass_kernel_spmd(
        nc, in_maps[:n_cores], core_ids=list(range(n_cores)), trace=trace
    )
    colmap = struct["colmap"]
    y = np.empty((n_cores * NPC, HID_DIM), np.float32)
    for k in range(n_cores):
        y[k * NPC : (k + 1) * NPC] = res.results[k]["yt"][:, colmap[k]].T
    return y, res


def kernel(h, src, dst, W, b, gamma, beta):
    y, _ = build_and_run(
        dict(h=h, src=src, dst=dst, W=W, b=b, gamma=gamma, beta=beta)
    )
    return y


# revision 22
# speedup vs baseline: 3.3983x; 1.0209x over previous
"""GCN layer (message passing + linear + BatchNorm) on 8 Trainium2 NeuronCores.

Strategy
--------
* Nodes are sharded across the 8 cores (12500 each) and bin-packed per core
  into C chunks of <=128 nodes such that each (chunk, src-class) edge bucket
  holds <= 512 edges (exactly 4 x 128-edge tiles).  Edges are partitioned by
  dst core so segment_sum is local; h is replicated.
* h is pre-split on host into bf16 (hi, lo) pairs packed as h2[N,128] bf16 so
  one gathered 256B row reconstructs the fp32 value exactly.
* dma_gather uses int16 indices (max 32767), so src ids are classed by range
  (4 classes of 25000 rows).  One gather call per (4-chunk batch, class) on
  its own SWDGE queue (4 queues -> 4x parallel Q7 descriptor generation).
* Per 128-edge tile, segment-sum is one TensorE matmul:
      psum_aggT[128 f', 128 n] += G2-as-lhsT @ S^T[e,n]
  where S^T[e,n] = (dst_local[e] == n) is built by VectorE is_equal against
  an iota row, and G2 is the gathered tile.
* z^T = W2^T @ aggT with W2 = [W; W] stacked (folds the hi+lo sum into the
  linear layer).  The bias b cancels in BatchNorm and is dropped.
* BN stats: ScalarE activation accum_out gives per-feature sums of z and z^2;
  a 1KB AllReduce combines the 8 cores; normalization is a per-partition
  (feature) tensor_scalar over z^T.  Output is written as y^T [128, C*128]
  per core; the host inverse-permutes and concatenates.
"""

import numpy as np
import ml_dtypes
from contextlib import ExitStack

import concourse.bass as bass
import concourse.tile as tile
import concourse.mybir as mybir
from concourse import bacc
from concourse.bass_utils import run_bass_kernel_spmd


def _install_ntff_hook():
    """Provide antenv.axon_hooks (absent on this image) so trace=True works."""
    import sys
    import types

    if "antenv.axon_hooks" in sys.modules:
        return
    mod = types.ModuleType("antenv.axon_hooks")
    holder = [None]
    mod.set_axon_ntff_profile_hook = lambda h: holder.__setitem__(0, h)
    mod.get_axon_ntff_profile_hook = lambda: holder[0]
    sys.modules["antenv.axon_hooks"] = mod
    try:
        from trn_agent_boot.trn_boot import _ntff_profile_via_ctypes

        hook = _ntff_profile_via_ctypes("/opt/axon/libaxon_pjrt.so")
        if hook is not None:
            mod.set_axon_ntff_profile_hook(hook)
    except Exception:
        pass


_install_ntff_hook()

BF16 = ml_dtypes.bfloat16

N_NODES = 100000
N_EDGES = 1600000
IN_DIM = 64
HID_DIM = 128
BN_EPS = 1e-5

CORES = 8
NPC = N_NODES // CORES            # 12500 nodes per core
NCLS = 4                          # src index classes (int16 gather limit)
CLS_SZ = N_NODES // NCLS          # 25000
TPB = 4                           # tiles (of 128 edges) per (chunk, class)
BUCKET_CAP = TPB * 128            # 512 edges per (chunk, class)
BATCH = 8                         # chunks per gather batch

_compiled = {}


def _pack_chunks(deg4, node_cap=128):
    """Next-fit pack nodes into chunks with per-class edge cap and node cap.

    deg4: [n_nodes, 4] per-class in-degree.  Returns (chunk_id, slot_id, C).
    """
    n = deg4.shape[0]
    chunk_id = np.zeros(n, np.int64)
    slot_id = np.zeros(n, np.int64)
    cur = 0
    fill = [0, 0, 0, 0]
    nodes_in = 0
    for i in range(n):
        d = deg4[i]
        if (
            nodes_in >= node_cap
            or fill[0] + d[0] > BUCKET_CAP
            or fill[1] + d[1] > BUCKET_CAP
            or fill[2] + d[2] > BUCKET_CAP
            or fill[3] + d[3] > BUCKET_CAP
        ):
            cur += 1
            fill = [0, 0, 0, 0]
            nodes_in = 0
        chunk_id[i] = cur
        slot_id[i] = nodes_in
        fill[0] += d[0]
        fill[1] += d[1]
        fill[2] += d[2]
        fill[3] += d[3]
        nodes_in += 1
    return chunk_id, slot_id, cur + 1


def _host_prep(h, src, dst, W, gamma, beta):
    h = np.ascontiguousarray(np.asarray(h, dtype=np.float32))
    src = np.asarray(src, dtype=np.int64)
    dst = np.asarray(dst, dtype=np.int64)

    hi = h.astype(BF16)
    lo = (h - hi.astype(np.float32)).astype(BF16)
    h2 = np.concatenate([hi, lo], axis=1)            # [N, 128] bf16

    core = dst // NPC
    cls = src // CLS_SZ
    idx16 = (src - cls * CLS_SZ).astype(np.int16)

    # per-node per-class in-degree -> per-core chunk packing
    deg4 = np.bincount(dst * NCLS + cls, minlength=N_NODES * NCLS).reshape(
        N_NODES, NCLS
    )
    chunk_all = np.zeros(N_NODES, np.int64)
    slot_all = np.zeros(N_NODES, np.int64)
    Cs = []
    for k in range(CORES):
        sl = slice(k * NPC, (k + 1) * NPC)
        c_id, s_id, Ck = _pack_chunks(deg4[sl])
        chunk_all[sl] = c_id
        slot_all[sl] = s_id
        Cs.append(Ck)
    C = int(max(Cs))

    chunk = chunk_all[dst]
    dst_local = slot_all[dst].astype(np.float32)

    # rank of each edge within its (core, cls, chunk) bucket
    key = (core * NCLS + cls) * C + chunk
    order = np.argsort(key, kind="stable")
    sk = key[order]
    grp_starts = np.r_[0, np.flatnonzero(np.diff(sk)) + 1]
    grp_sizes = np.diff(np.r_[grp_starts, len(sk)])
    rank = np.arange(len(sk)) - np.repeat(grp_starts, grp_sizes)
    assert rank.max() < BUCKET_CAP

    e_core = core[order]
    e_cls = cls[order]
    e_chunk = chunk[order]
    pos = e_chunk * BUCKET_CAP + rank            # class-stream position
    t_edge = rank >> 7
    p_edge = pos & 127
    col_edge = e_chunk * (NCLS * TPB) + e_cls * TPB + t_edge

    L = C * BUCKET_CAP                           # positions per class stream
    Dtot = C * NCLS * TPB

    idx_w = []
    for g in range(NCLS):
        arr = np.zeros((CORES, L), dtype=np.int16)
        m = e_cls == g
        arr[e_core[m], pos[m]] = idx16[order][m]
        w = arr.reshape(CORES, L // 16, 16).transpose(0, 2, 1)
        idx_w.append(np.ascontiguousarray(np.tile(w, (1, 8, 1))))

    dstl = np.full((CORES, 128, Dtot), -1.0, dtype=BF16)
    dstl[e_core, p_edge, col_edge] = dst_local[order].astype(BF16)

    W2 = np.concatenate([np.asarray(W, np.float32)] * 2, axis=0)  # [128, 128]
    # iota tiled NJ times along free dim: [128, NJ*128] (removes the
    # broadcast AP on the is_equal input)
    nj = NCLS * TPB
    iota = np.ascontiguousarray(
        np.broadcast_to(
            np.tile(np.arange(128, dtype=np.float32).astype(BF16), nj),
            (128, nj * 128),
        )
    )
    g128 = np.asarray(gamma, np.float32).reshape(HID_DIM, 1)
    b128 = np.asarray(beta, np.float32).reshape(HID_DIM, 1)

    in_maps = []
    for k in range(CORES):
        m = {
            "h2": h2,
            "dstl": np.ascontiguousarray(dstl[k]),
            "w2": W2,
            "gammap": g128,
            "betap": b128,
            "iotap": iota,
        }
        for g in range(NCLS):
            m[f"idx{g}"] = idx_w[g][k]
        in_maps.append(m)

    colmap = (chunk_all * 128 + slot_all).reshape(CORES, NPC)
    struct = dict(C=C, L=L, Dtot=Dtot, colmap=colmap)
    return in_maps, struct


def _build(struct, n_cores=CORES, use_collective=True, skip_gather=False,
           skip_compute=False, single_packet=False, n_queues=4):
    C = struct["C"]
    L = struct["L"]
    Dtot = struct["Dtot"]
    ncols_out = C * 128

    f32 = mybir.dt.float32
    bf16 = mybir.dt.bfloat16
    i16 = mybir.dt.int16
    AF = mybir.ActivationFunctionType
    OP = mybir.AluOpType

    nc = bacc.Bacc("TRN2", debug=False, num_swdge_queues=n_queues)

    h2_t = nc.dram_tensor("h2", [N_NODES, 128], bf16, kind="ExternalInput")
    idx_t = [
        nc.dram_tensor(f"idx{g}", [128, L // 16], i16, kind="ExternalInput")
        for g in range(NCLS)
    ]
    dstl_t = nc.dram_tensor("dstl", [128, Dtot], bf16, kind="ExternalInput")
    w2_t = nc.dram_tensor("w2", [128, 128], f32, kind="ExternalInput")
    gamma_t = nc.dram_tensor("gammap", [128, 1], f32, kind="ExternalInput")
    beta_t = nc.dram_tensor("betap", [128, 1], f32, kind="ExternalInput")
    NJ = NCLS * TPB  # sub-tiles (matmuls) per chunk = 16
    iota_t = nc.dram_tensor("iotap", [128, NJ * 128], bf16, kind="ExternalInput")
    yt_t = nc.dram_tensor("yt", [128, ncols_out], f32, kind="ExternalOutput")

    with tile.TileContext(nc) as tc, ExitStack() as ctx:
        const = ctx.enter_context(tc.tile_pool(name="const", bufs=1))
        zpool = ctx.enter_context(tc.tile_pool(name="zpool", bufs=1))
        gpools = [
            ctx.enter_context(tc.tile_pool(name=f"gp{g}", bufs=2)) for g in range(NCLS)
        ]
        spool = ctx.enter_context(tc.tile_pool(name="spool", bufs=3))
        apool = ctx.enter_context(tc.tile_pool(name="apool", bufs=2))
        sqpool = ctx.enter_context(tc.tile_pool(name="sqpool", bufs=2))
        stat = ctx.enter_context(tc.tile_pool(name="stat", bufs=1))
        pa_pool = ctx.enter_context(tc.tile_pool(name="pa", bufs=2, space="PSUM"))
        pz_pool = ctx.enter_context(tc.tile_pool(name="pz", bufs=2, space="PSUM"))
        dram = ctx.enter_context(tc.tile_pool(name="dram", bufs=1, space="DRAM"))

        iota_sb = const.tile([128, NJ * 128], bf16)
        nc.sync.dma_start(iota_sb[:], iota_t[:])
        w2_sb = const.tile([128, 128], f32)
        nc.sync.dma_start(w2_sb[:], w2_t[:])
        gamma_sb = const.tile([128, 1], f32)
        nc.sync.dma_start(gamma_sb[:], gamma_t[:])
        beta_sb = const.tile([128, 1], f32)
        nc.sync.dma_start(beta_sb[:], beta_t[:])
        dstl_sb = const.tile([128, Dtot], bf16)
        nc.sync.dma_start(dstl_sb[:], dstl_t[:])
        idx_sb = []
        for g in range(NCLS):
            t = const.tile([128, L // 16], i16, name=f"idxsb{g}")
            nc.sync.dma_start(t[:], idx_t[g][:])
            idx_sb.append(t)

        zt = zpool.tile([128, ncols_out], f32)
        s1c = stat.tile([128, C], f32)
        s2c = stat.tile([128, C], f32)
        if skip_compute:
            nc.vector.memset(zt[:], 0.0)
            nc.vector.memset(s1c[:], 0.0)
            nc.vector.memset(s2c[:], 1.0)

        for b0 in range(0, C, BATCH):
            cs = list(range(b0, min(C, b0 + BATCH)))
            nb = len(cs)
            gts = []
            for g in range(NCLS):
                cols = nb * TPB
                gt = gpools[g].tile([128, BATCH * TPB, 128], bf16, name=f"gt{g}")
                pos0 = b0 * BUCKET_CAP
                npos = cols * 128
                if skip_gather:
                    nc.vector.memset(gt[:, :cols, :], 0.0)
                else:
                    nc.gpsimd.dma_gather(
                        gt[:, :cols, :],
                        h2_t[g * CLS_SZ : (g + 1) * CLS_SZ, :],
                        idx_sb[g][:, pos0 // 16 : (pos0 + npos) // 16],
                        npos,
                        npos,
                        128,
                        single_packet=single_packet,
                        queue_num=g % n_queues,
                    )
                gts.append(gt)

            if skip_compute:
                continue
            for c in cs:
                s_t = spool.tile([128, NJ, 128], bf16, name="s_t")
                d_sl = dstl_sb[:, c * NJ : (c + 1) * NJ]
                nc.vector.tensor_tensor(
                    s_t[:],
                    iota_sb[:].rearrange("p (j n) -> p j n", n=128),
                    d_sl.unsqueeze(2).broadcast_to([128, NJ, 128]),
                    OP.is_equal,
                )
                pa = pa_pool.tile([128, 128], f32, name="pa")
                j = 0
                for g in range(NCLS):
                    for t_i in range(TPB):
                        nc.tensor.matmul(
                            pa[:],
                            gts[g][:, (c - b0) * TPB + t_i, :],
                            s_t[:, j, :],
                            start=(j == 0),
                            stop=(j == NJ - 1),
                        )
                        j += 1
                agg_sb = apool.tile([128, 128], f32, name="agg_sb")
                nc.scalar.copy(agg_sb[:], pa[:])
                pz = pz_pool.tile([128, 128], f32, name="pz")
                nc.tensor.matmul(pz[:], w2_sb[:], agg_sb[:], start=True, stop=True)
                nc.scalar.activation(
                    zt[:, c * 128 : (c + 1) * 128], pz[:], AF.Copy,
                    accum_out=s1c[:, c : c + 1],
                )
                sq = sqpool.tile([128, 128], f32, name="sq")
                nc.scalar.activation(
                    sq[:], pz[:], AF.Square, accum_out=s2c[:, c : c + 1],
                )

        # global BN stats
        ccin_sb = stat.tile([128, 2], f32)
        nc.vector.tensor_reduce(ccin_sb[:, 0:1], s1c[:], mybir.AxisListType.X, OP.add)
        nc.vector.tensor_reduce(ccin_sb[:, 1:2], s2c[:], mybir.AxisListType.X, OP.add)
        ccsb = stat.tile([128, 2], f32)
        if n_cores > 1 and use_collective:
            cc_in = dram.tile([128, 2], f32)
            cc_out = dram.tile([128, 2], f32)
            nc.sync.dma_start(cc_in[:], ccin_sb[:])
            nc.gpsimd.collective_compute(
                "AllReduce",
                OP.add,
                replica_groups=[list(range(n_cores))],
                ins=[cc_in.opt()],
                outs=[cc_out.opt()],
            )
            nc.sync.dma_start(ccsb[:], cc_out[:])
        else:
            nc.vector.tensor_copy(ccsb[:], ccin_sb[:])

        n_total = float(N_NODES) if (n_cores > 1 and use_collective) else float(NPC)
        mean = stat.tile([128, 1], f32)
        nc.vector.tensor_scalar_mul(mean[:], ccsb[:, 0:1], 1.0 / n_total)
        ex2 = stat.tile([128, 1], f32)
        nc.vector.tensor_scalar_mul(ex2[:], ccsb[:, 1:2], 1.0 / n_total)
        msq = stat.tile([128, 1], f32)
        nc.vector.tensor_tensor(msq[:], mean[:], mean[:], OP.mult)
        varep = stat.tile([128, 1], f32)
        nc.vector.tensor_tensor(varep[:], ex2[:], msq[:], OP.subtract)
        nc.vector.tensor_scalar_add(varep[:], varep[:], BN_EPS)
        rvar = stat.tile([128, 1], f32)
        nc.vector.reciprocal(rvar[:], varep[:])
        rstd = stat.tile([128, 1], f32)
        nc.scalar.sqrt(rstd[:], rvar[:])
        a_sc = stat.tile([128, 1], f32)
        nc.vector.tensor_tensor(a_sc[:], gamma_sb[:], rstd[:], OP.mult)
        ma = stat.tile([128, 1], f32)
        nc.vector.tensor_tensor(ma[:], mean[:], a_sc[:], OP.mult)
        c_sc = stat.tile([128, 1], f32)
        nc.vector.tensor_tensor(c_sc[:], beta_sb[:], ma[:], OP.subtract)

        # normalize + store in 4 column blocks (overlap DVE with output DMA)
        nblk = (ncols_out + 3) // 4
        for i0 in range(0, ncols_out, nblk):
            i1 = min(ncols_out, i0 + nblk)
            nc.vector.tensor_scalar(
                zt[:, i0:i1], zt[:, i0:i1], a_sc[:, 0:1], c_sc[:, 0:1],
                OP.mult, OP.add,
            )
            nc.sync.dma_start(yt_t[:, i0:i1], zt[:, i0:i1])

    # Tile assigns SWDGE completion-sem lanes (DMASW{i}) round-robin in
    # scheduled order, and each lane is locked to one SWDGE queue at first
    # use.  Rewrite each gather's queue to lane % n_queues so the mapping is
    # consistent (keeps the 4-way parallel Q7 descriptor generation).
    import re

    for blk in nc.m.functions[0].blocks:
        for inst in blk.instructions:
            if isinstance(inst, mybir.InstDMAGatherAnt):
                si = inst.sync_info
                if si is None or not si.on_update:
                    continue
                m = re.match(r"DMASW(\d+)", si.on_update[0].ant_name or "")
                if m:
                    inst.queue_num = int(m.group(1)) % n_queues

    nc.compile()
    return nc


def build_and_run(inputs, trace=False, n_cores=CORES, **build_kw):
    in_maps, struct = _host_prep(
        inputs["h"], inputs["src"], inputs["dst"], inputs["W"],
        inputs["gamma"], inputs["beta"],
    )
    key = (struct["C"], n_cores, tuple(sorted(build_kw.items())))
    if key not in _compiled:
        _compiled[key] = _build(struct, n_cores=n_cores, **build_kw)
    nc = _compiled[key]
    res = run_bass_kernel_spmd(
        nc, in_maps[:n_cores], core_ids=list(range(n_cores)), trace=trace
    )
    colmap = struct["colmap"]
    y = np.empty((n_cores * NPC, HID_DIM), np.float32)
    for k in range(n_cores):
        y[k * NPC : (k + 1) * NPC] = res.results[k]["yt"][:, colmap[k]].T
    return y, res


def kernel(h, src, dst, W, b, gamma, beta):
    y, _ = build_and_run(
        dict(h=h, src=src, dst=dst, W=W, b=b, gamma=gamma, beta=beta)
    )
    return y


# revision 23
# speedup vs baseline: 3.7467x; 1.1025x over previous
"""GCN layer (message passing + linear + BatchNorm) on 8 Trainium2 NeuronCores.

Strategy
--------
* Nodes are sharded across the 8 cores (12500 each) and bin-packed per core
  into C chunks of <=128 nodes such that each (chunk, src-class) edge bucket
  holds <= 512 edges (exactly 4 x 128-edge tiles).  Edges are partitioned by
  dst core so segment_sum is local; h is replicated.
* h is pre-split on host into bf16 (hi, lo) pairs packed as h2[N,128] bf16 so
  one gathered 256B row reconstructs the fp32 value exactly.
* dma_gather uses int16 indices (max 32767), so src ids are classed by range
  (4 classes of 25000 rows).  One gather call per (4-chunk batch, class) on
  its own SWDGE queue (4 queues -> 4x parallel Q7 descriptor generation).
* Per 128-edge tile, segment-sum is one TensorE matmul:
      psum_aggT[128 f', 128 n] += G2-as-lhsT @ S^T[e,n]
  where S^T[e,n] = (dst_local[e] == n) is built by VectorE is_equal against
  an iota row, and G2 is the gathered tile.
* z^T = W2^T @ aggT with W2 = [W; W] stacked (folds the hi+lo sum into the
  linear layer).  The bias b cancels in BatchNorm and is dropped.
* BN stats: ScalarE activation accum_out gives per-feature sums of z and z^2;
  a 1KB AllReduce combines the 8 cores; normalization is a per-partition
  (feature) tensor_scalar over z^T.  Output is written as y^T [128, C*128]
  per core; the host inverse-permutes and concatenates.
"""

import numpy as np
import ml_dtypes
from contextlib import ExitStack

import concourse.bass as bass
import concourse.tile as tile
import concourse.mybir as mybir
from concourse import bacc
from concourse.bass_utils import run_bass_kernel_spmd


def _install_ntff_hook():
    """Provide antenv.axon_hooks (absent on this image) so trace=True works."""
    import sys
    import types

    if "antenv.axon_hooks" in sys.modules:
        return
    mod = types.ModuleType("antenv.axon_hooks")
    holder = [None]
    mod.set_axon_ntff_profile_hook = lambda h: holder.__setitem__(0, h)
    mod.get_axon_ntff_profile_hook = lambda: holder[0]
    sys.modules["antenv.axon_hooks"] = mod
    try:
        from trn_agent_boot.trn_boot import _ntff_profile_via_ctypes

        hook = _ntff_profile_via_ctypes("/opt/axon/libaxon_pjrt.so")
        if hook is not None:
            mod.set_axon_ntff_profile_hook(hook)
    except Exception:
        pass


_install_ntff_hook()

BF16 = ml_dtypes.bfloat16

N_NODES = 100000
N_EDGES = 1600000
IN_DIM = 64
HID_DIM = 128
BN_EPS = 1e-5

CORES = 8
NPC = N_NODES // CORES            # 12500 nodes per core
NCLS = 4                          # src index classes (int16 gather limit)
CLS_SZ = N_NODES // NCLS          # 25000
TPB = 4                           # tiles (of 128 edges) per (chunk, class)
BUCKET_CAP = TPB * 128            # 512 edges per (chunk, class)
BATCH = 4                         # chunks per gather batch

_compiled = {}


def _pack_chunks(deg4, node_cap=128):
    """Next-fit pack nodes into chunks with per-class edge cap and node cap.

    deg4: [n_nodes, 4] per-class in-degree.  Returns (chunk_id, slot_id, C).
    """
    n = deg4.shape[0]
    chunk_id = np.zeros(n, np.int64)
    slot_id = np.zeros(n, np.int64)
    cur = 0
    fill = [0, 0, 0, 0]
    nodes_in = 0
    for i in range(n):
        d = deg4[i]
        if (
            nodes_in >= node_cap
            or fill[0] + d[0] > BUCKET_CAP
            or fill[1] + d[1] > BUCKET_CAP
            or fill[2] + d[2] > BUCKET_CAP
            or fill[3] + d[3] > BUCKET_CAP
        ):
            cur += 1
            fill = [0, 0, 0, 0]
            nodes_in = 0
        chunk_id[i] = cur
        slot_id[i] = nodes_in
        fill[0] += d[0]
        fill[1] += d[1]
        fill[2] += d[2]
        fill[3] += d[3]
        nodes_in += 1
    return chunk_id, slot_id, cur + 1


def _host_prep(h, src, dst, W, gamma, beta):
    h = np.ascontiguousarray(np.asarray(h, dtype=np.float32))
    src = np.asarray(src, dtype=np.int64)
    dst = np.asarray(dst, dtype=np.int64)

    hi = h.astype(BF16)
    lo = (h - hi.astype(np.float32)).astype(BF16)
    h2 = np.concatenate([hi, lo], axis=1)            # [N, 128] bf16

    core = dst // NPC
    cls = src // CLS_SZ
    idx16 = (src - cls * CLS_SZ).astype(np.int16)

    # per-node per-class in-degree -> per-core chunk packing
    deg4 = np.bincount(dst * NCLS + cls, minlength=N_NODES * NCLS).reshape(
        N_NODES, NCLS
    )
    chunk_all = np.zeros(N_NODES, np.int64)
    slot_all = np.zeros(N_NODES, np.int64)
    Cs = []
    for k in range(CORES):
        sl = slice(k * NPC, (k + 1) * NPC)
        c_id, s_id, Ck = _pack_chunks(deg4[sl])
        chunk_all[sl] = c_id
        slot_all[sl] = s_id
        Cs.append(Ck)
    C = int(max(Cs))

    chunk = chunk_all[dst]
    dst_local = slot_all[dst].astype(np.float32)

    # rank of each edge within its (core, cls, chunk) bucket
    key = (core * NCLS + cls) * C + chunk
    order = np.argsort(key, kind="stable")
    sk = key[order]
    grp_starts = np.r_[0, np.flatnonzero(np.diff(sk)) + 1]
    grp_sizes = np.diff(np.r_[grp_starts, len(sk)])
    rank = np.arange(len(sk)) - np.repeat(grp_starts, grp_sizes)
    assert rank.max() < BUCKET_CAP

    e_core = core[order]
    e_cls = cls[order]
    e_chunk = chunk[order]
    pos = e_chunk * BUCKET_CAP + rank            # class-stream position
    t_edge = rank >> 7
    p_edge = pos & 127
    col_edge = e_chunk * (NCLS * TPB) + e_cls * TPB + t_edge

    L = C * BUCKET_CAP                           # positions per class stream
    Dtot = C * NCLS * TPB

    idx_w = []
    for g in range(NCLS):
        arr = np.zeros((CORES, L), dtype=np.int16)
        m = e_cls == g
        arr[e_core[m], pos[m]] = idx16[order][m]
        w = arr.reshape(CORES, L // 16, 16).transpose(0, 2, 1)
        idx_w.append(np.ascontiguousarray(np.tile(w, (1, 8, 1))))

    dstl = np.full((CORES, 128, Dtot), -1.0, dtype=BF16)
    dstl[e_core, p_edge, col_edge] = dst_local[order].astype(BF16)

    W2 = np.concatenate([np.asarray(W, np.float32)] * 2, axis=0)  # [128, 128]
    # iota tiled NJ times along free dim: [128, NJ*128] (removes the
    # broadcast AP on the is_equal input)
    nj = NCLS * TPB
    iota = np.ascontiguousarray(
        np.broadcast_to(
            np.tile(np.arange(128, dtype=np.float32).astype(BF16), nj),
            (128, nj * 128),
        )
    )
    g128 = np.asarray(gamma, np.float32).reshape(HID_DIM, 1)
    b128 = np.asarray(beta, np.float32).reshape(HID_DIM, 1)

    in_maps = []
    for k in range(CORES):
        m = {
            "h2": h2,
            "dstl": np.ascontiguousarray(dstl[k]),
            "w2": W2,
            "gammap": g128,
            "betap": b128,
            "iotap": iota,
        }
        for g in range(NCLS):
            m[f"idx{g}"] = idx_w[g][k]
        in_maps.append(m)

    colmap = (chunk_all * 128 + slot_all).reshape(CORES, NPC)
    struct = dict(C=C, L=L, Dtot=Dtot, colmap=colmap)
    return in_maps, struct


def _build(struct, n_cores=CORES, use_collective=True, skip_gather=False,
           skip_compute=False, single_packet=False, n_queues=4):
    C = struct["C"]
    L = struct["L"]
    Dtot = struct["Dtot"]
    ncols_out = C * 128

    f32 = mybir.dt.float32
    bf16 = mybir.dt.bfloat16
    i16 = mybir.dt.int16
    AF = mybir.ActivationFunctionType
    OP = mybir.AluOpType

    nc = bacc.Bacc("TRN2", debug=False, num_swdge_queues=n_queues)

    h2_t = nc.dram_tensor("h2", [N_NODES, 128], bf16, kind="ExternalInput")
    idx_t = [
        nc.dram_tensor(f"idx{g}", [128, L // 16], i16, kind="ExternalInput")
        for g in range(NCLS)
    ]
    dstl_t = nc.dram_tensor("dstl", [128, Dtot], bf16, kind="ExternalInput")
    w2_t = nc.dram_tensor("w2", [128, 128], f32, kind="ExternalInput")
    gamma_t = nc.dram_tensor("gammap", [128, 1], f32, kind="ExternalInput")
    beta_t = nc.dram_tensor("betap", [128, 1], f32, kind="ExternalInput")
    NJ = NCLS * TPB  # sub-tiles (matmuls) per chunk = 16
    iota_t = nc.dram_tensor("iotap", [128, NJ * 128], bf16, kind="ExternalInput")
    yt_t = nc.dram_tensor("yt", [128, ncols_out], f32, kind="ExternalOutput")

    with tile.TileContext(nc) as tc, ExitStack() as ctx:
        const = ctx.enter_context(tc.tile_pool(name="const", bufs=1))
        zpool = ctx.enter_context(tc.tile_pool(name="zpool", bufs=1))
        gpools = [
            ctx.enter_context(tc.tile_pool(name=f"gp{g}", bufs=3)) for g in range(NCLS)
        ]
        spool = ctx.enter_context(tc.tile_pool(name="spool", bufs=4))
        apool = ctx.enter_context(tc.tile_pool(name="apool", bufs=2))
        sqpool = ctx.enter_context(tc.tile_pool(name="sqpool", bufs=2))
        stat = ctx.enter_context(tc.tile_pool(name="stat", bufs=1))
        pa_pool = ctx.enter_context(tc.tile_pool(name="pa", bufs=2, space="PSUM"))
        pz_pool = ctx.enter_context(tc.tile_pool(name="pz", bufs=2, space="PSUM"))
        dram = ctx.enter_context(tc.tile_pool(name="dram", bufs=1, space="DRAM"))

        iota_sb = const.tile([128, NJ * 128], bf16)
        nc.sync.dma_start(iota_sb[:], iota_t[:])
        w2_sb = const.tile([128, 128], f32)
        nc.sync.dma_start(w2_sb[:], w2_t[:])
        gamma_sb = const.tile([128, 1], f32)
        nc.sync.dma_start(gamma_sb[:], gamma_t[:])
        beta_sb = const.tile([128, 1], f32)
        nc.sync.dma_start(beta_sb[:], beta_t[:])
        dstl_sb = const.tile([128, Dtot], bf16)
        nc.sync.dma_start(dstl_sb[:], dstl_t[:])
        idx_sb = []
        for g in range(NCLS):
            t = const.tile([128, L // 16], i16, name=f"idxsb{g}")
            nc.sync.dma_start(t[:], idx_t[g][:])
            idx_sb.append(t)

        zt = zpool.tile([128, ncols_out], f32)
        s1c = stat.tile([128, C], f32)
        s2c = stat.tile([128, C], f32)
        if skip_compute:
            nc.vector.memset(zt[:], 0.0)
            nc.vector.memset(s1c[:], 0.0)
            nc.vector.memset(s2c[:], 1.0)

        for b0 in range(0, C, BATCH):
            cs = list(range(b0, min(C, b0 + BATCH)))
            nb = len(cs)
            gts = []
            for g in range(NCLS):
                cols = nb * TPB
                gt = gpools[g].tile([128, BATCH * TPB, 128], bf16, name=f"gt{g}")
                pos0 = b0 * BUCKET_CAP
                npos = cols * 128
                if skip_gather:
                    nc.vector.memset(gt[:, :cols, :], 0.0)
                else:
                    nc.gpsimd.dma_gather(
                        gt[:, :cols, :],
                        h2_t[g * CLS_SZ : (g + 1) * CLS_SZ, :],
                        idx_sb[g][:, pos0 // 16 : (pos0 + npos) // 16],
                        npos,
                        npos,
                        128,
                        single_packet=single_packet,
                        queue_num=g % n_queues,
                    )
                gts.append(gt)

            if skip_compute:
                continue
            for c in cs:
                s_t = spool.tile([128, NJ, 128], bf16, name="s_t")
                d_sl = dstl_sb[:, c * NJ : (c + 1) * NJ]
                nc.vector.tensor_tensor(
                    s_t[:],
                    iota_sb[:].rearrange("p (j n) -> p j n", n=128),
                    d_sl.unsqueeze(2).broadcast_to([128, NJ, 128]),
                    OP.is_equal,
                )
                pa = pa_pool.tile([128, 128], f32, name="pa")
                j = 0
                for g in range(NCLS):
                    for t_i in range(TPB):
                        nc.tensor.matmul(
                            pa[:],
                            gts[g][:, (c - b0) * TPB + t_i, :],
                            s_t[:, j, :],
                            start=(j == 0),
                            stop=(j == NJ - 1),
                        )
                        j += 1
                agg_sb = apool.tile([128, 128], f32, name="agg_sb")
                nc.scalar.copy(agg_sb[:], pa[:])
                pz = pz_pool.tile([128, 128], f32, name="pz")
                nc.tensor.matmul(pz[:], w2_sb[:], agg_sb[:], start=True, stop=True)
                nc.scalar.activation(
                    zt[:, c * 128 : (c + 1) * 128], pz[:], AF.Copy,
                    accum_out=s1c[:, c : c + 1],
                )
                sq = sqpool.tile([128, 128], f32, name="sq")
                nc.scalar.activation(
                    sq[:], pz[:], AF.Square, accum_out=s2c[:, c : c + 1],
                )

        # global BN stats
        ccin_sb = stat.tile([128, 2], f32)
        nc.vector.tensor_reduce(ccin_sb[:, 0:1], s1c[:], mybir.AxisListType.X, OP.add)
        nc.vector.tensor_reduce(ccin_sb[:, 1:2], s2c[:], mybir.AxisListType.X, OP.add)
        ccsb = stat.tile([128, 2], f32)
        if n_cores > 1 and use_collective:
            cc_in = dram.tile([128, 2], f32)
            cc_out = dram.tile([128, 2], f32)
            nc.sync.dma_start(cc_in[:], ccin_sb[:])
            nc.gpsimd.collective_compute(
                "AllReduce",
                OP.add,
                replica_groups=[list(range(n_cores))],
                ins=[cc_in.opt()],
                outs=[cc_out.opt()],
            )
            nc.sync.dma_start(ccsb[:], cc_out[:])
        else:
            nc.vector.tensor_copy(ccsb[:], ccin_sb[:])

        n_total = float(N_NODES) if (n_cores > 1 and use_collective) else float(NPC)
        mean = stat.tile([128, 1], f32)
        nc.vector.tensor_scalar_mul(mean[:], ccsb[:, 0:1], 1.0 / n_total)
        ex2 = stat.tile([128, 1], f32)
        nc.vector.tensor_scalar_mul(ex2[:], ccsb[:, 1:2], 1.0 / n_total)
        msq = stat.tile([128, 1], f32)
        nc.vector.tensor_tensor(msq[:], mean[:], mean[:], OP.mult)
        varep = stat.tile([128, 1], f32)
        nc.vector.tensor_tensor(varep[:], ex2[:], msq[:], OP.subtract)
        nc.vector.tensor_scalar_add(varep[:], varep[:], BN_EPS)
        rvar = stat.tile([128, 1], f32)
        nc.vector.reciprocal(rvar[:], varep[:])
        rstd = stat.tile([128, 1], f32)
        nc.scalar.sqrt(rstd[:], rvar[:])
        a_sc = stat.tile([128, 1], f32)
        nc.vector.tensor_tensor(a_sc[:], gamma_sb[:], rstd[:], OP.mult)
        ma = stat.tile([128, 1], f32)
        nc.vector.tensor_tensor(ma[:], mean[:], a_sc[:], OP.mult)
        c_sc = stat.tile([128, 1], f32)
        nc.vector.tensor_tensor(c_sc[:], beta_sb[:], ma[:], OP.subtract)

        # normalize + store in 4 column blocks (overlap DVE with output DMA)
        nblk = (ncols_out + 3) // 4
        for i0 in range(0, ncols_out, nblk):
            i1 = min(ncols_out, i0 + nblk)
            nc.vector.tensor_scalar(
                zt[:, i0:i1], zt[:, i0:i1], a_sc[:, 0:1], c_sc[:, 0:1],
                OP.mult, OP.add,
            )
            nc.sync.dma_start(yt_t[:, i0:i1], zt[:, i0:i1])

    # Tile assigns SWDGE completion-sem lanes (DMASW{i}) round-robin in
    # scheduled order, and each lane is locked to one SWDGE queue at first
    # use.  Rewrite each gather's queue to lane % n_queues so the mapping is
    # consistent (keeps the 4-way parallel Q7 descriptor generation).
    import re

    for blk in nc.m.functions[0].blocks:
        for inst in blk.instructions:
            if isinstance(inst, mybir.InstDMAGatherAnt):
                si = inst.sync_info
                if si is None or not si.on_update:
                    continue
                m = re.match(r"DMASW(\d+)", si.on_update[0].ant_name or "")
                if m:
                    inst.queue_num = int(m.group(1)) % n_queues

    nc.compile()
    return nc


def build_and_run(inputs, trace=False, n_cores=CORES, **build_kw):
    in_maps, struct = _host_prep(
        inputs["h"], inputs["src"], inputs["dst"], inputs["W"],
        inputs["gamma"], inputs["beta"],
    )
    key = (struct["C"], n_cores, tuple(sorted(build_kw.items())))
    if key not in _compiled:
        _compiled[key] = _build(struct, n_cores=n_cores, **build_kw)
    nc = _compiled[key]
    res = run_bass_kernel_spmd(
        nc, in_maps[:n_cores], core_ids=list(range(n_cores)), trace=trace
    )
    colmap = struct["colmap"]
    y = np.empty((n_cores * NPC, HID_DIM), np.float32)
    for k in range(n_cores):
        y[k * NPC : (k + 1) * NPC] = res.results[k]["yt"][:, colmap[k]].T
    return y, res


def kernel(h, src, dst, W, b, gamma, beta):
    y, _ = build_and_run(
        dict(h=h, src=src, dst=dst, W=W, b=b, gamma=gamma, beta=beta)
    )
    return y
